# revision 46
# baseline (speedup 1.0000x reference)
"""Trainium2 Bass/Tile kernel for EntropyRecyclingLanguageNet (vq_codebook).

Computes, for x[B,D]:
    pw    = softmax(x @ attn_w + attn_b)               # [B,P]
    rec   = pw @ pattern_dict                          # [B,D]
    par   = rec @ self_w + self_b - rec                # [B,D]
    out   = (rec * sigmoid(||par||)) @ out_w + out_b   # [B,V]

Sharding: tensor-parallel over the vocab dim (V=32000 -> 4000 per core);
every core runs the full small stage for all B rows (cheap), and the
dominant cost -- writing the [8192, 4000] output slice -- is spread
across the 8 cores.  Host gathers with a concat along axis 1.

The kernel is memory-bound on the output write, so the whole heavy data
path runs in fp16 (tolerance is 2e-2; measured fp16 rel err ~4e-4):
fp16 output halves HBM write traffic vs f32.

Key structure (per core):
  * weight folds on host (input-independent):
      m2  = pattern_dict @ out_w                        [P, V] (sharded)
      m3x = [pattern_dict @ (self_w - I) + self_b | 1]  [P, D+1]
    (self_b folds into every row because sum_p ewT[p,b] = denom[b];
     out_b is added on the host during the gather -- typically zero.)
  * phase A per 512-wide block: logitsT = attn_w.T @ xT on PE; ACT Exp
    with attn_b bias gives unnormalized expwT rows of ewTd.
  * phase A2 per tile: ONE K=64 matmul ewT_tile.T @ m3x yields
    parScaled = denom*par (cols 0:D) and the denominator (col D);
    q = ||parScaled||^2 via a DVE copy/square/accumulate chain.
  * gate per group of 8 tiles, ACT functions limited to {Exp, Ln} so a
    single table set (natural_log_exp_and_others, forced below) serves
    the whole kernel:  scl = sigmoid(sqrt(q)/denom)/denom.
  * phase B: out_tile = ewT_tile.T @ m2.  The PE HAM clock gate keeps
    the array at 1.2 GHz for this kernel's duty cycle, so K=64 matmuls
    are row-tiled: batch tile i occupies array rows 0:64 and tile i+32
    rows 64:128 (operands duplicated at partition offset 64), and the
    two matmuls stream concurrently -> ~2x effective PE throughput.
    The per-row scale scl is applied during the PSUM->SBUF drain (ACT
    activation-with-scale / DVE tensor_scalar, split ~57/43); fp16
    SBUF tiles DMA straight out.
"""

import numpy as np

import concourse.bass as bass
import concourse.mybir as mybir
import concourse.tile as tile
from concourse import bacc
from concourse.bass_utils import run_bass_kernel_spmd

# The greedy act-table pass picks the first set containing each function
# (Exp -> exp_and_others, Ln -> natural_log), which thrash (~17 reloads,
# 2.7us each).  Every ACT function this kernel uses (exp, ln, square,
# identity, copy, memset_zero) lives in natural_log_exp_and_others, so
# blank out every other set: the pass then emits exactly one table load.
# Set ids stay aligned with act_info.json (only values are emptied).
_KEEP_SET = "natural_log_exp_and_others"
_orig_gat = bacc.get_activation_tables


def _single_set_tables(arch):
    return {k: (v if k == _KEEP_SET else set()) for k, v in _orig_gat(arch).items()}


bacc.get_activation_tables = _single_set_tables

B, D, P, V = 8192, 128, 64, 32000
NCORES = 8
VS = V // NCORES        # vocab cols per core (4000)
BT = 128                # batch tile (partition dim)
NBT = B // BT           # 64 batch tiles
HBT = NBT // 2          # 32: tile i pairs with tile i+32 (array rows hi)
G = 8                   # batch tiles per gate group
NG = NBT // G           # 8 groups; groups 0-3 lo half, 4-7 hi half
NG2 = NG // 2           # 4 pair-groups
W = 512                 # phase-A block width (4 batch tiles)
PCH = 1024              # PSUM proj tile width (2 banks; 2 matmuls each)
ACT_FRAC = 0.57         # fraction of projection drains on the scalar engine
F16 = mybir.dt.float16
F32 = mybir.dt.float32
AF = mybir.ActivationFunctionType

_cache = {}


def _build():
    nc = bacc.Bacc(
        "TRN2",
        target_bir_lowering=False,
        debug=False,
        num_devices=NCORES,
    )

    d_xT = nc.dram_tensor("xT", [D, B], F16, kind="ExternalInput").ap()
    d_attn_w = nc.dram_tensor("attn_w", [D, P], F16, kind="ExternalInput").ap()
    d_attn_b = nc.dram_tensor("attn_b", [P, 1], F32, kind="ExternalInput").ap()
    d_m3x = nc.dram_tensor("m3x", [P, D + 1], F16, kind="ExternalInput").ap()
    d_m2 = nc.dram_tensor("m2", [P, VS], F16, kind="ExternalInput").ap()
    d_out = nc.dram_tensor("out", [B, VS], F16, kind="ExternalOutput").ap()

    with tile.TileContext(nc) as tc:
        with (
            tc.tile_pool(name="consts", bufs=1) as cpool,
            tc.tile_pool(name="grp", bufs=8) as gpool,
            tc.tile_pool(name="small", bufs=3) as spool,
            tc.tile_pool(name="stage", bufs=6) as stpool,
            tc.tile_pool(name="pss", bufs=1, space="PSUM") as pss,
            tc.tile_pool(name="pso", bufs=3, space="PSUM") as pso,
        ):
            # ---- resident constants (small first so block 0 starts early)
            attn_w = cpool.tile([D, P], F16)
            nc.sync.dma_start(attn_w[:], d_attn_w[:])
            attn_b = cpool.tile([P, 1], F32)
            nc.sync.dma_start(attn_b[:], d_attn_b[:])
            # m3x / m2 live twice: rows 0:P for lo batch tiles (array rows
            # 0:64) and rows P:2P for hi batch tiles (rows 64:128)
            m3x = cpool.tile([2 * P, D + 1], F16)
            nc.sync.dma_start(m3x[0:P, :], d_m3x[:])
            nc.sync.dma_start(m3x[P:2 * P, :], d_m3x[:])

            xT = cpool.tile([D, B], F16)
            m2 = cpool.tile([2 * P, VS], F16)
            for c in range(8):  # chunked so batch tile 0 can start early
                nc.sync.dma_start(
                    xT[:, c * (B // 8):(c + 1) * (B // 8)],
                    d_xT[:, c * (B // 8):(c + 1) * (B // 8)],
                )
                if c == 0:
                    nc.sync.dma_start(m2[0:P, :], d_m2[:])
                    nc.sync.dma_start(m2[P:2 * P, :], d_m2[:])

            # PE HAM warm-up: one dense zero-gap accumulation burst buys a
            # few warm microseconds at the start; cheap, kept.
            ps_warm = pso.tile([128, W], F32, tag="o", name="ps_warm")
            NWU = 16
            for wu in range(NWU):
                nc.tensor.matmul(
                    ps_warm[0:P, :], attn_w[:], xT[:, 0:W],
                    start=(wu == 0), stop=(wu == NWU - 1),
                )

            # unnormalized softmax numerators, transposed; batch tile i
            # lives at rows (i//32)*64 : +64, cols (i%32)*128 : +128
            ewTd = cpool.tile([2 * P, B // 2], F16)

            scls = {}
            drain_st = [0.0]

            def phase_a1(g):
                # logits + exp for one group, W-wide blocks
                half = g // NG2
                r0 = half * P
                cb0 = (g % NG2) * G * BT
                for blk in range(G * BT // W):
                    xc = g * G * BT + blk * W
                    ps_lg = pss.tile([P, W], F32, tag="lg", name=f"ps_lg_{g}_{blk}")
                    nc.tensor.matmul(
                        ps_lg[:], attn_w[:], xT[:, xc:xc + W],
                        start=True, stop=True,
                    )
                    nc.scalar.activation(
                        ewTd[r0:r0 + P, cb0 + blk * W:cb0 + (blk + 1) * W],
                        ps_lg[:], AF.Exp, bias=attn_b[:],
                    )

            def phase_a2_pair(pg):
                # parScaled (self_b folded into m3x rows) plus denominator
                # column, K=64; the lo tile lands in bank 0 and the hi tile
                # in bank 1 of one slot, streaming concurrently via PE
                # row-tiling.  Then both halves' gates.
                qd = {}
                for half in range(2):
                    g = pg + half * NG2
                    qd[half] = (
                        gpool.tile([BT, G], F32, tag=f"qall{half}", name=f"qall_{g}"),
                        gpool.tile([BT, G], F32, tag=f"dall{half}", name=f"dall_{g}"),
                    )
                for tg in range(G):
                    iL = pg * G + tg
                    cb = iL * BT
                    for half in range(2):
                        qall, dall = qd[half]
                        r0 = half * P
                        i = iL + half * HBT
                        ps_pd = pss.tile([BT, D + 1], F32, tag="pd",
                                         name=f"ps_pd_{i}")
                        nc.tensor.matmul(
                            ps_pd[:], ewTd[r0:r0 + P, cb:cb + BT],
                            m3x[r0:r0 + P, :],
                            start=True, stop=True,
                        )
                        # q = ||parScaled||^2: fp16 copy out of PSUM (DVE),
                        # square on the idle GpSimd, accumulate on DVE
                        pdsb = spool.tile([BT, D], F16, tag="pdsb",
                                          name=f"pdsb_{i}")
                        nc.vector.tensor_copy(pdsb[:], ps_pd[:, 0:D])
                        sq = spool.tile([BT, D], F16, tag="sq", name=f"sq_{i}")
                        nc.gpsimd.tensor_mul(sq[:], pdsb[:], pdsb[:])
                        sqj = spool.tile([BT, D], F16, tag="sqj", name=f"sqj_{i}")
                        nc.vector.tensor_scalar(
                            sqj[:], sq[:], 1.0, 0.0, mybir.AluOpType.mult,
                            mybir.AluOpType.add, accum_out=qall[:, tg:tg + 1],
                        )
                        nc.vector.tensor_copy(
                            dall[:, tg:tg + 1], ps_pd[:, D:D + 1]
                        )

                # gate: scl = sigmoid(sqrt(q)/d)/d with Ln/Exp only
                for half in range(2):
                    g = pg + half * NG2
                    qall, dall = qd[half]
                    rd = gpool.tile([BT, G], F32, tag="rd", name=f"rd_{g}")
                    nc.vector.reciprocal(rd[:], dall[:])
                    lnq = gpool.tile([BT, G], F32, tag="lnq", name=f"lnq_{g}")
                    nc.scalar.activation(lnq[:], qall[:], AF.Ln)
                    smag = gpool.tile([BT, G], F32, tag="smag", name=f"smag_{g}")
                    nc.scalar.activation(smag[:], lnq[:], AF.Exp, scale=0.5)
                    mag = gpool.tile([BT, G], F32, tag="mag", name=f"mag_{g}")
                    nc.vector.tensor_mul(mag[:], smag[:], rd[:])
                    emn = gpool.tile([BT, G], F32, tag="emn", name=f"emn_{g}")
                    nc.scalar.activation(emn[:], mag[:], AF.Exp, scale=-1.0)
                    sp1 = gpool.tile([BT, G], F32, tag="sp1", name=f"sp1_{g}")
                    nc.vector.tensor_scalar_add(sp1[:], emn[:], 1.0)
                    sig = gpool.tile([BT, G], F32, tag="sig", name=f"sig_{g}")
                    nc.vector.reciprocal(sig[:], sp1[:])
                    scl = gpool.tile([BT, G], F32, tag="scl", name=f"scl_{g}")
                    nc.vector.tensor_mul(scl[:], sig[:], rd[:])
                    scls[g] = scl

            def drain(dst, src, sc):
                drain_st[0] += ACT_FRAC
                if drain_st[0] >= 1.0:
                    drain_st[0] -= 1.0
                    nc.scalar.activation(dst, src, AF.Identity, scale=sc)
                else:
                    nc.vector.tensor_scalar_mul(dst, src, sc)

            def phase_b_pair(pg):
                # tiles iLo (rows 0:64) and iLo+32 (rows 64:128) project
                # concurrently via PE row-tiling; scale in the drain
                scl_lo, scl_hi = scls[pg], scls[pg + NG2]
                for tg in range(G):
                    iL = pg * G + tg
                    iH = iL + HBT
                    cb = iL * BT
                    scL = scl_lo[:, tg:tg + 1]
                    scH = scl_hi[:, tg:tg + 1]
                    obL = stpool.tile([BT, VS], F16, tag="ob", name=f"ob_{iL}")
                    obH = stpool.tile([BT, VS], F16, tag="ob", name=f"ob_{iH}")
                    for jv in range(VS // PCH + 1):
                        w = min(PCH, VS - jv * PCH)
                        psL = pso.tile([BT, PCH], F32, tag="o", name=f"psL_{iL}_{jv}")
                        psH = pso.tile([BT, PCH], F32, tag="o", name=f"psH_{iH}_{jv}")
                        h0 = 0
                        while h0 < w:
                            hw = min(512, w - h0)
                            off = jv * PCH + h0
                            nc.tensor.matmul(
                                psL[:, h0:h0 + hw],
                                ewTd[0:P, cb:cb + BT], m2[0:P, off:off + hw],
                                start=True, stop=True,
                            )
                            nc.tensor.matmul(
                                psH[:, h0:h0 + hw],
                                ewTd[P:2 * P, cb:cb + BT],
                                m2[P:2 * P, off:off + hw],
                                start=True, stop=True,
                            )
                            h0 += hw
                        dL = obL[:, jv * PCH:jv * PCH + w]
                        dH = obH[:, jv * PCH:jv * PCH + w]
                        drain(dL, psL[:, 0:w], scL)
                        drain(dH, psH[:, 0:w], scH)
                    nc.sync.dma_start(d_out[iL * BT:(iL + 1) * BT, :], obL[:])
                    nc.sync.dma_start(d_out[iH * BT:(iH + 1) * BT, :], obH[:])

            # software pipeline: each pair-group's A phases (exp for both
            # halves, paired par matmuls, gates) run one pair-group ahead
            # of its projection
            phase_a1(0)
            phase_a1(NG2)
            phase_a2_pair(0)
            for pg in range(NG2):
                if pg + 1 < NG2:
                    phase_a1(pg + 1)
                    phase_a1(pg + 1 + NG2)
                    phase_a2_pair(pg + 1)
                phase_b_pair(pg)

            # single reader so the warm-up matmuls aren't dangling
            wu_junk = spool.tile([128, W], F16, tag="wuj", name="wu_junk")
            nc.vector.tensor_copy(wu_junk[:], ps_warm[:])

    nc.compile()
    return nc


def _get_nc():
    if "nc" not in _cache:
        _cache["nc"] = _build()
    return _cache["nc"]


def make_in_maps(x, pattern_dict, attn_w, attn_b, self_w, self_b, out_w, out_b):
    x = np.asarray(x, dtype=np.float32)
    pattern_dict = np.asarray(pattern_dict, dtype=np.float32)
    attn_w = np.asarray(attn_w, dtype=np.float32)
    attn_b = np.asarray(attn_b, dtype=np.float32)
    self_w = np.asarray(self_w, dtype=np.float32)
    self_b = np.asarray(self_b, dtype=np.float32)
    out_w = np.asarray(out_w, dtype=np.float32)
    out_b = np.asarray(out_b, dtype=np.float32)

    # self_b folds into every row of m1: with unnormalized weights ewT,
    # sum_p ewT[p,b]*(m1[p,:] + self_b) = denom[b]*par[b,:] exactly.
    m3x = np.zeros((P, D + 1), dtype=np.float32)
    m3x[:, 0:D] = pattern_dict @ (self_w - np.eye(D, dtype=np.float32)) + self_b
    m3x[:, D] = 1.0
    m2 = pattern_dict @ out_w  # [P, V]

    shared = {
        "xT": np.ascontiguousarray(x.T).astype(np.float16),
        "attn_w": attn_w.astype(np.float16),
        "attn_b": np.ascontiguousarray(attn_b.reshape(P, 1)),
        "m3x": m3x.astype(np.float16),
    }
    in_maps = []
    for c in range(NCORES):
        m = dict(shared)
        m["m2"] = np.ascontiguousarray(m2[:, c * VS:(c + 1) * VS]).astype(np.float16)
        in_maps.append(m)
    return in_maps


def kernel(x, pattern_dict, attn_w, attn_b, self_w, self_b, out_w, out_b):
    in_maps = make_in_maps(
        x, pattern_dict, attn_w, attn_b, self_w, self_b, out_w, out_b
    )
    nc = _get_nc()
    res = run_bass_kernel_spmd(nc, in_maps, list(range(NCORES)))
    out = np.concatenate(
        [res.results[c]["out"].astype(np.float32) for c in range(NCORES)], axis=1
    )
    out_b = np.asarray(out_b, dtype=np.float32)
    if np.any(out_b):
        out += out_b
    return out


# revision 49
# speedup vs baseline: 1.1098x; 1.1098x over previous
"""Trainium2 Bass/Tile kernel for EntropyRecyclingLanguageNet (vq_codebook).

Computes, for x[B,D]:
    pw    = softmax(x @ attn_w + attn_b)               # [B,P]
    rec   = pw @ pattern_dict                          # [B,D]
    par   = rec @ self_w + self_b - rec                # [B,D]
    out   = (rec * sigmoid(||par||)) @ out_w + out_b   # [B,V]

Sharding: tensor-parallel over the vocab dim (V=32000 -> 4000 per core);
every core runs the full small stage for all B rows (cheap), and the
dominant cost -- writing the [8192, 4000] output slice -- is spread
across the 8 cores.  Host gathers with a concat along axis 1.

The kernel is memory-bound on the output write, so the whole heavy data
path runs in fp16 (tolerance is 2e-2; measured fp16 rel err ~4e-4):
fp16 output halves HBM write traffic vs f32.

Key structure (per core):
  * weight folds on host (input-independent):
      m2  = pattern_dict @ out_w                        [P, V] (sharded)
      m3x = [pattern_dict @ (self_w - I) + self_b | 1]  [P, D+1]
    (self_b folds into every row because sum_p ewT[p,b] = denom[b];
     out_b is added on the host during the gather -- typically zero.)
  * phase A per 512-wide block: logitsT = attn_w.T @ xT on PE; ACT Exp
    with attn_b bias gives unnormalized expwT rows of ewTd.
  * phase A2 per tile: ONE K=64 matmul ewT_tile.T @ m3x yields
    parScaled = denom*par (cols 0:D) and the denominator (col D);
    q = ||parScaled||^2 via a DVE copy/square/accumulate chain.
  * gate per group of 8 tiles, ACT functions limited to {Exp, Ln} so a
    single table set (natural_log_exp_and_others, forced below) serves
    the whole kernel:  scl = sigmoid(sqrt(q)/denom)/denom.
  * phase B: out_tile = ewT_tile.T @ m2.  The PE HAM clock gate keeps
    the array at 1.2 GHz for this kernel's duty cycle, so K=64 matmuls
    are row-tiled: batch tile i occupies array rows 0:64 and tile i+32
    rows 64:128 (operands duplicated at partition offset 64), and the
    two matmuls stream concurrently -> ~2x effective PE throughput.
    The per-row scale scl is applied during the PSUM->SBUF drain (ACT
    activation-with-scale / DVE tensor_scalar, split ~57/43); fp16
    SBUF tiles DMA straight out.
"""

import numpy as np

import concourse.bass as bass
import concourse.mybir as mybir
import concourse.tile as tile
from concourse import bacc
from concourse.bass_utils import run_bass_kernel_spmd

# The greedy act-table pass picks the first set containing each function
# (Exp -> exp_and_others, Ln -> natural_log), which thrash (~17 reloads,
# 2.7us each).  Every ACT function this kernel uses (exp, ln, square,
# identity, copy, memset_zero) lives in natural_log_exp_and_others, so
# blank out every other set: the pass then emits exactly one table load.
# Set ids stay aligned with act_info.json (only values are emptied).
_KEEP_SET = "natural_log_exp_and_others"
_orig_gat = bacc.get_activation_tables


def _single_set_tables(arch):
    return {k: (v if k == _KEEP_SET else set()) for k, v in _orig_gat(arch).items()}


bacc.get_activation_tables = _single_set_tables

B, D, P, V = 8192, 128, 64, 32000
NCORES = 8
VS = V // NCORES        # vocab cols per core (4000)
BT = 128                # batch tile (partition dim)
NBT = B // BT           # 64 batch tiles
HBT = NBT // 2          # 32: tile i pairs with tile i+32 (array rows hi)
G = 8                   # batch tiles per gate group
NG = NBT // G           # 8 groups; groups 0-3 lo half, 4-7 hi half
W = 512                 # phase-A block width (4 batch tiles)
PCH = 1024              # PSUM proj tile width (2 banks; 2 matmuls each)
ACT_FRAC = 0.57         # fraction of projection drains on the scalar engine
F16 = mybir.dt.float16
F32 = mybir.dt.float32
AF = mybir.ActivationFunctionType

_cache = {}


def _build():
    nc = bacc.Bacc(
        "TRN2",
        target_bir_lowering=False,
        debug=False,
        num_devices=NCORES,
    )

    d_xT = nc.dram_tensor("xT", [D, B], F16, kind="ExternalInput").ap()
    d_attn_w = nc.dram_tensor("attn_w", [D, P], F16, kind="ExternalInput").ap()
    d_attn_b = nc.dram_tensor("attn_b", [P, 1], F32, kind="ExternalInput").ap()
    d_m3x = nc.dram_tensor("m3x", [P, D + 1], F16, kind="ExternalInput").ap()
    d_m2 = nc.dram_tensor("m2", [P, VS], F16, kind="ExternalInput").ap()
    d_out = nc.dram_tensor("out", [B, VS], F16, kind="ExternalOutput").ap()

    with tile.TileContext(nc) as tc:
        with (
            tc.tile_pool(name="consts", bufs=1) as cpool,
            tc.tile_pool(name="grp", bufs=8) as gpool,
            tc.tile_pool(name="small", bufs=4) as spool,
            tc.tile_pool(name="stage", bufs=6) as stpool,
            tc.tile_pool(name="pso", bufs=4, space="PSUM") as pso,
        ):
            # ---- resident constants (small first so block 0 starts early)
            attn_w = cpool.tile([D, P], F16)
            nc.sync.dma_start(attn_w[:], d_attn_w[:])
            attn_b = cpool.tile([P, 1], F32)
            nc.sync.dma_start(attn_b[:], d_attn_b[:])
            # m3x / m2 live twice: rows 0:P for lo batch tiles (array rows
            # 0:64) and rows P:2P for hi batch tiles (rows 64:128)
            m3x = cpool.tile([2 * P, D + 1], F16)
            nc.sync.dma_start(m3x[0:P, :], d_m3x[:])
            nc.sync.dma_start(m3x[P:2 * P, :], d_m3x[:])

            xT = cpool.tile([D, B], F16)
            m2 = cpool.tile([2 * P, VS], F16)
            for c in range(16):  # chunked so batch tile 0 can start early
                nc.sync.dma_start(
                    xT[:, c * (B // 16):(c + 1) * (B // 16)],
                    d_xT[:, c * (B // 16):(c + 1) * (B // 16)],
                )
                if c == 0:
                    nc.sync.dma_start(m2[0:P, :], d_m2[:])
                    nc.sync.dma_start(m2[P:2 * P, :], d_m2[:])

            # PE HAM warm-up: one dense zero-gap accumulation burst buys a
            # few warm microseconds at the start; cheap, kept.
            ps_warm = pso.tile([128, W], F32, tag="o", name="ps_warm")
            NWU = 16
            for wu in range(NWU):
                nc.tensor.matmul(
                    ps_warm[0:P, :], attn_w[:], xT[:, 0:W],
                    start=(wu == 0), stop=(wu == NWU - 1),
                )

            # unnormalized softmax numerators, transposed; batch tile i
            # lives at rows (i//32)*64 : +64, cols (i%32)*128 : +128
            ewTd = cpool.tile([2 * P, B // 2], F16)

            scls = {}
            drain_st = [0.0]

            def phase_a(g):
                half = g // NG2
                r0 = half * P
                cb0 = (g % NG2) * G * BT

                # logits + exp, W-wide blocks
                for blk in range(G * BT // W):
                    xc = g * G * BT + blk * W
                    ps_lg = pso.tile([P, W], F32, tag="o", name=f"ps_lg_{g}_{blk}")
                    nc.tensor.matmul(
                        ps_lg[:], attn_w[:], xT[:, xc:xc + W],
                        start=True, stop=True,
                    )
                    nc.scalar.activation(
                        ewTd[r0:r0 + P, cb0 + blk * W:cb0 + (blk + 1) * W],
                        ps_lg[:], AF.Exp, bias=attn_b[:],
                    )

                dall = gpool.tile([BT, G], F32, tag="dall", name=f"dall_{g}")
                qall = gpool.tile([BT, G], F32, tag="qall", name=f"qall_{g}")

                # parScaled (self_b folded into m3x rows) plus the
                # denominator column (m3x's trailing ones column), K=64
                for tg in range(G):
                    i = g * G + tg
                    cb = (i % HBT) * BT
                    ps_pd = pso.tile([BT, D + 1], F32, tag="o", name=f"ps_pd_{i}")
                    nc.tensor.matmul(
                        ps_pd[:], ewTd[r0:r0 + P, cb:cb + BT], m3x[r0:r0 + P, :],
                        start=True, stop=True,
                    )
                    # q = ||parScaled||^2 on DVE: fp16 copy out of PSUM,
                    # 2x fp16 square, tensor_scalar accumulate
                    pdsb = spool.tile([BT, D], F16, tag="pdsb", name=f"pdsb_{i}")
                    nc.vector.tensor_copy(pdsb[:], ps_pd[:, 0:D])
                    sq = spool.tile([BT, D], F16, tag="sq", name=f"sq_{i}")
                    nc.vector.tensor_mul(sq[:], pdsb[:], pdsb[:])
                    sqj = spool.tile([BT, D], F16, tag="sqj", name=f"sqj_{i}")
                    nc.vector.tensor_scalar(
                        sqj[:], sq[:], 1.0, 0.0, mybir.AluOpType.mult,
                        mybir.AluOpType.add, accum_out=qall[:, tg:tg + 1],
                    )
                    nc.vector.tensor_copy(dall[:, tg:tg + 1], ps_pd[:, D:D + 1])

                # gate: scl = sigmoid(sqrt(q)/d)/d with Ln/Exp only
                rd = gpool.tile([BT, G], F32, tag="rd", name=f"rd_{g}")
                nc.vector.reciprocal(rd[:], dall[:])
                lnq = gpool.tile([BT, G], F32, tag="lnq", name=f"lnq_{g}")
                nc.scalar.activation(lnq[:], qall[:], AF.Ln)
                smag = gpool.tile([BT, G], F32, tag="smag", name=f"smag_{g}")
                nc.scalar.activation(smag[:], lnq[:], AF.Exp, scale=0.5)
                mag = gpool.tile([BT, G], F32, tag="mag", name=f"mag_{g}")
                nc.vector.tensor_mul(mag[:], smag[:], rd[:])
                emn = gpool.tile([BT, G], F32, tag="emn", name=f"emn_{g}")
                nc.scalar.activation(emn[:], mag[:], AF.Exp, scale=-1.0)
                sp1 = gpool.tile([BT, G], F32, tag="sp1", name=f"sp1_{g}")
                nc.vector.tensor_scalar_add(sp1[:], emn[:], 1.0)
                sig = gpool.tile([BT, G], F32, tag="sig", name=f"sig_{g}")
                nc.vector.reciprocal(sig[:], sp1[:])
                scl = gpool.tile([BT, G], F32, tag="scl", name=f"scl_{g}")
                nc.vector.tensor_mul(scl[:], sig[:], rd[:])
                scls[g] = scl

            def drain(dst, src, sc):
                drain_st[0] += ACT_FRAC
                if drain_st[0] >= 1.0:
                    drain_st[0] -= 1.0
                    nc.scalar.activation(dst, src, AF.Identity, scale=sc)
                else:
                    nc.vector.tensor_scalar_mul(dst, src, sc)

            def phase_b_pair(pg):
                # tiles iLo (rows 0:64) and iLo+32 (rows 64:128) project
                # concurrently via PE row-tiling; scale in the drain
                scl_lo, scl_hi = scls[pg], scls[pg + NG2]
                for tg in range(G):
                    iL = pg * G + tg
                    iH = iL + HBT
                    cb = iL * BT
                    scL = scl_lo[:, tg:tg + 1]
                    scH = scl_hi[:, tg:tg + 1]
                    obL = stpool.tile([BT, VS], F16, tag="ob", name=f"ob_{iL}")
                    obH = stpool.tile([BT, VS], F16, tag="ob", name=f"ob_{iH}")
                    for jv in range(VS // PCH + 1):
                        w = min(PCH, VS - jv * PCH)
                        psL = pso.tile([BT, PCH], F32, tag="o", name=f"psL_{iL}_{jv}")
                        psH = pso.tile([BT, PCH], F32, tag="o", name=f"psH_{iH}_{jv}")
                        h0 = 0
                        while h0 < w:
                            hw = min(512, w - h0)
                            off = jv * PCH + h0
                            nc.tensor.matmul(
                                psL[:, h0:h0 + hw],
                                ewTd[0:P, cb:cb + BT], m2[0:P, off:off + hw],
                                start=True, stop=True,
                            )
                            nc.tensor.matmul(
                                psH[:, h0:h0 + hw],
                                ewTd[P:2 * P, cb:cb + BT],
                                m2[P:2 * P, off:off + hw],
                                start=True, stop=True,
                            )
                            h0 += hw
                        dL = obL[:, jv * PCH:jv * PCH + w]
                        dH = obH[:, jv * PCH:jv * PCH + w]
                        drain(dL, psL[:, 0:w], scL)
                        drain(dH, psH[:, 0:w], scH)
                    nc.sync.dma_start(d_out[iL * BT:(iL + 1) * BT, :], obL[:])
                    nc.sync.dma_start(d_out[iH * BT:(iH + 1) * BT, :], obH[:])

            # software pipeline: the lo/hi gate groups of each pair run
            # ahead of the pair's projection
            NG2 = NG // 2
            phase_a(0)
            phase_a(NG2)
            for pg in range(NG2):
                if pg + 1 < NG2:
                    phase_a(pg + 1)
                    phase_a(pg + 1 + NG2)
                phase_b_pair(pg)

            # single reader so the warm-up matmuls aren't dangling
            wu_junk = spool.tile([128, W], F16, tag="wuj", name="wu_junk")
            nc.vector.tensor_copy(wu_junk[:], ps_warm[:])

    nc.compile()
    return nc


def _get_nc():
    if "nc" not in _cache:
        _cache["nc"] = _build()
    return _cache["nc"]


def make_in_maps(x, pattern_dict, attn_w, attn_b, self_w, self_b, out_w, out_b):
    x = np.asarray(x, dtype=np.float32)
    pattern_dict = np.asarray(pattern_dict, dtype=np.float32)
    attn_w = np.asarray(attn_w, dtype=np.float32)
    attn_b = np.asarray(attn_b, dtype=np.float32)
    self_w = np.asarray(self_w, dtype=np.float32)
    self_b = np.asarray(self_b, dtype=np.float32)
    out_w = np.asarray(out_w, dtype=np.float32)
    out_b = np.asarray(out_b, dtype=np.float32)

    # self_b folds into every row of m1: with unnormalized weights ewT,
    # sum_p ewT[p,b]*(m1[p,:] + self_b) = denom[b]*par[b,:] exactly.
    m3x = np.zeros((P, D + 1), dtype=np.float32)
    m3x[:, 0:D] = pattern_dict @ (self_w - np.eye(D, dtype=np.float32)) + self_b
    m3x[:, D] = 1.0
    m2 = pattern_dict @ out_w  # [P, V]

    shared = {
        "xT": np.ascontiguousarray(x.T).astype(np.float16),
        "attn_w": attn_w.astype(np.float16),
        "attn_b": np.ascontiguousarray(attn_b.reshape(P, 1)),
        "m3x": m3x.astype(np.float16),
    }
    in_maps = []
    for c in range(NCORES):
        m = dict(shared)
        m["m2"] = np.ascontiguousarray(m2[:, c * VS:(c + 1) * VS]).astype(np.float16)
        in_maps.append(m)
    return in_maps


def kernel(x, pattern_dict, attn_w, attn_b, self_w, self_b, out_w, out_b):
    in_maps = make_in_maps(
        x, pattern_dict, attn_w, attn_b, self_w, self_b, out_w, out_b
    )
    nc = _get_nc()
    res = run_bass_kernel_spmd(nc, in_maps, list(range(NCORES)))
    out = np.concatenate(
        [res.results[c]["out"].astype(np.float32) for c in range(NCORES)], axis=1
    )
    out_b = np.asarray(out_b, dtype=np.float32)
    if np.any(out_b):
        out += out_b
    return out
